# revision 5
# baseline (speedup 1.0000x reference)
"""MoLoRA linear kernel for Trainium2 (8 NeuronCores, SPMD data-parallel).

Computes: out = x @ W.T + alpha * (per-token top-2 routed LoRA)
Sharding: tokens (B*S = 4096) split 8 ways; all weights replicated.

Numerics: single-pass fp16 everywhere (inputs rounded to fp16, fp32 PSUM
accumulation, fp16 output writeback). Measured against an fp64 reference
this gives ~3.2e-4 rel-RMS error (tolerance 2e-2): fp16 rounding of the
operands dominates; top-2 expert selection flips on ~1/4096 tokens with
negligible output effect. Renormalized top-2 softmax == sigmoid of the
top-2 logit gap.

Layout: every DMA is contiguous per partition (host pre-tiles all operands
into [partition, ...] blocks). The base matmul streams W through the PE
(x chunks stationary); the LoRA-up matmul opens or closes each PSUM
accumulation group so its result lands in the same banks for free.

Self-contained: needs numpy + the concourse (bass) stack importable
(falls back to /opt/trn_rl_repo).
"""

import sys

import numpy as np

try:
    import concourse.bass as bass  # noqa: F401
except Exception:  # pragma: no cover
    sys.path.insert(0, "/opt/trn_rl_repo")

import concourse.bacc as bacc
import concourse.mybir as mybir
import concourse.tile as tile
from concourse import bass_utils
from concourse.masks import make_identity

F32 = mybir.dt.float32
F16 = mybir.dt.float16
AX = mybir.AxisListType.X
OP = mybir.AluOpType

# Problem shapes (hardcoded per contract)
B, S, H, O, E, R = 2, 2048, 2048, 2048, 8, 16
ER = E * R            # 128 = stacked lora rank dim, exactly one partition dim
GA = ER + E           # 136 = lora-A cols + gate cols, fused moving operand
TOKENS = B * S        # 4096
NCORES = 8
T = TOKENS // NCORES  # 512 tokens per core
P = 128
KT = H // P           # 16 contraction chunks
NQ = 4                # output quarters
OQ = O // NQ          # 512 cols per quarter = one PSUM bank
NTC = T // P          # 4 token chunks of 128
KC = 4                # ga matmuls trail base by this many k-chunks
LORA_ALPHA = 16.0
NEG_BIG = 1.0e30
IORD = (2, 3, 0, 1)   # token-chunk processing order (router frees pb6/7 first)


def _build_nc(unroll=1):
    """Build the per-core bass program (identical on all 8 cores).

    unroll>1 repeats the whole computation in one NEFF (same inputs,
    same outputs) — used only for steady-state timing measurements."""
    nc = bacc.Bacc(None, target_bir_lowering=False, debug=False)

    xh = nc.dram_tensor("xh", [P, KT, T], F16, kind="ExternalInput")
    whq = nc.dram_tensor("whq", [NQ, P, KT, OQ], F16, kind="ExternalInput")
    gah = nc.dram_tensor("gah", [P, KT, GA], F16, kind="ExternalInput")
    bcat = nc.dram_tensor("bcat", [ER, O], F16, kind="ExternalInput")
    out = nc.dram_tensor("out", [NQ, P, NTC, OQ], F16, kind="ExternalOutput")

    whq_r = whq[:, :, :, :]

    with tile.TileContext(nc) as tc:
        with (
            tc.tile_pool(name="const", bufs=1) as const_pool,
            tc.tile_pool(name="big", bufs=1) as big_pool,
            tc.tile_pool(name="wstream", bufs=2) as w_pool,
            tc.tile_pool(name="ostage", bufs=2) as o_pool,
            tc.tile_pool(name="router", bufs=1) as r_pool,
            tc.tile_pool(name="psum", bufs=1, space="PSUM") as pp,
        ):
            identity = const_pool.tile([P, P], F16)
            make_identity(nc, identity)

            def body(it):
                # ---- resident loads (ACT ring; weight stream uses SP) ----
                xh_sb = big_pool.tile([P, KT, T], F16, name=f"xh_sb{it}",
                                      tag="xh_sb")
                nc.scalar.dma_start(out=xh_sb[:, 0:2, :], in_=xh[:, 0:2, :])
                gah_sb = big_pool.tile([P, KT, GA], F16, name=f"gah_sb{it}",
                                       tag="gah_sb")
                nc.scalar.dma_start(out=gah_sb[:], in_=gah[:, :, :])
                nc.scalar.dma_start(out=xh_sb[:, 2:4, :], in_=xh[:, 2:4, :])
                nc.scalar.dma_start(out=xh_sb[:, 4:8, :], in_=xh[:, 4:8, :])
                nc.scalar.dma_start(out=xh_sb[:, 8:16, :], in_=xh[:, 8:16, :])
                bcat_sb = big_pool.tile([P, O], F16, name=f"bcat_sb{it}",
                                        tag="bcat_sb")
                nc.scalar.dma_start(out=bcat_sb[:], in_=bcat[:, :])

                twT_sb = big_pool.tile([P, T], F16, name=f"twT_sb{it}",
                                       tag="twT_sb")  # weighted lora-down

                def wq_load(q):
                    """Stream quarter q's weights into a resident tile."""
                    wq = w_pool.tile([P, KT, OQ], F16, name=f"wq{q}_{it}",
                                     tag="wq")
                    if q == 0:
                        chunks = [(0, 2), (2, 4), (4, 8), (8, 12), (12, 16)]
                    else:
                        chunks = [(0, 8), (8, 16)]
                    for lo, hi in chunks:
                        nc.sync.dma_start(out=wq[:, lo:hi, :],
                                          in_=whq_r[q, :, lo:hi, :])
                    return wq

                def quarter0(ga_tiles):
                    """O-quarter 0 (banks pb0-3) + the lora-down/logits
                    matmuls (pb4-7, trailing by KC so the gate tensor never
                    blocks the in-order PE queue during DMA rampup)."""
                    wq = wq_load(0)
                    accs = [
                        pp.tile([P, OQ], F32, name=f"acc0_{i}_{it}",
                                tag=f"pb{i}")
                        for i in range(NTC)
                    ]

                    def ga_mms(k):
                        for i in IORD:
                            ts = slice(i * P, (i + 1) * P)
                            nc.tensor.matmul(
                                ga_tiles[i][:], lhsT=xh_sb[:, k, ts],
                                rhs=gah_sb[:, k, :], start=(k == 0),
                                stop=(k == KT - 1),
                            )

                    for k in range(KT):
                        for i in range(NTC):
                            ts = slice(i * P, (i + 1) * P)
                            nc.tensor.matmul(
                                accs[i][:], lhsT=xh_sb[:, k, ts],
                                rhs=wq[:, k, :], start=(k == 0), stop=False,
                            )
                        if k >= KC:
                            ga_mms(k - KC)
                    for k in range(KT - KC, KT):
                        ga_mms(k)
                    return accs

                def base_quarter(q, tc_major=False, evict_per_tc=False):
                    """One O-quarter of the base matmul; banks alternate
                    between pb0-3 (even q) and pb4-7 (odd q). The lora
                    up-projection opens each accumulation group (twT is
                    ready by q1)."""
                    cols = slice(q * OQ, (q + 1) * OQ)
                    bank = (q % 2) * 4
                    wq = wq_load(q)
                    accs = {}
                    order = IORD if (tc_major or q == 1) else range(NTC)

                    def open_up(i):
                        accs[i] = pp.tile([P, OQ], F32,
                                          name=f"acc{q}_{i}_{it}",
                                          tag=f"pb{bank + i}")
                        ts = slice(i * P, (i + 1) * P)
                        nc.tensor.matmul(
                            accs[i][:], lhsT=twT_sb[:, ts],
                            rhs=bcat_sb[:, cols], start=True, stop=False,
                        )

                    if tc_major:
                        for i in order:
                            ts = slice(i * P, (i + 1) * P)
                            open_up(i)
                            for k in range(KT):
                                nc.tensor.matmul(
                                    accs[i][:], lhsT=xh_sb[:, k, ts],
                                    rhs=wq[:, k, :], start=False,
                                    stop=(k == KT - 1),
                                )
                            if evict_per_tc:
                                evict_tc(q, i, accs[i])
                    else:
                        for i in order:
                            open_up(i)
                        for k in range(KT):
                            for i in order:
                                ts = slice(i * P, (i + 1) * P)
                                nc.tensor.matmul(
                                    accs[i][:], lhsT=xh_sb[:, k, ts],
                                    rhs=wq[:, k, :], start=False,
                                    stop=(k == KT - 1),
                                )
                    return accs

                def up_close(q, accs):
                    """Close each accumulation group with the up matmul."""
                    for i in range(NTC):
                        ts = slice(i * P, (i + 1) * P)
                        nc.tensor.matmul(
                            accs[i][:], lhsT=twT_sb[:, ts],
                            rhs=bcat_sb[:, q * OQ : (q + 1) * OQ],
                            start=False, stop=True,
                        )

                def evict(q, accs):
                    """Cast accs to fp16, ship the quarter in one DMA."""
                    ost = o_pool.tile([P, NTC, OQ], F16, name=f"ost{q}_{it}",
                                      tag="ost")
                    for i in range(NTC):
                        nc.vector.tensor_copy(ost[:, i, :], accs[i][:])
                    nc.scalar.dma_start(out=out[q, :, :, :], in_=ost[:])

                def evict_tc(q, i, acc):
                    ost = o_pool.tile([P, OQ], F16, name=f"ost{q}_{i}_{it}",
                                      tag="ostc")
                    nc.vector.tensor_copy(ost[:], acc[:])
                    nc.scalar.dma_start(out=out[q, :, i, :], in_=ost[:])

                def router_math(ga_tiles):
                    """Batched top-2 routing for all 4 token chunks at once.
                    ga_tiles[i][:, ER:GA] are the logits [t=128, e=8]."""
                    l_all = r_pool.tile([P, NTC, E], F32, name=f"l_all{it}",
                                        tag="l_all")
                    for i in IORD:
                        nc.vector.tensor_copy(l_all[:, i, :],
                                              ga_tiles[i][:, ER:GA])
                    m1 = r_pool.tile([P, NTC], F32, name=f"m1{it}", tag="m1")
                    nc.vector.reduce_max(out=m1[:], in_=l_all[:], axis=AX)

                    def bcast(ap):  # [P, NTC] -> [P, NTC, E]
                        return ap.rearrange("p c -> p c ()").broadcast_to(
                            [P, NTC, E])

                    is1 = r_pool.tile([P, NTC, E], F32, name=f"is1{it}",
                                      tag="is1")
                    nc.vector.tensor_tensor(
                        out=is1[:], in0=l_all[:], in1=bcast(m1[:]),
                        op=OP.is_equal
                    )
                    l2 = r_pool.tile([P, NTC, E], F32, name=f"l2{it}",
                                     tag="l2")
                    nc.vector.tensor_scalar(
                        out=l2[:], in0=is1[:], scalar1=-NEG_BIG, scalar2=None,
                        op0=OP.mult,
                    )
                    nc.vector.tensor_add(out=l2[:], in0=l2[:], in1=l_all[:])
                    m2 = r_pool.tile([P, NTC], F32, name=f"m2{it}", tag="m2")
                    nc.vector.reduce_max(out=m2[:], in_=l2[:], axis=AX)
                    is2 = r_pool.tile([P, NTC, E], F32, name=f"is2{it}",
                                      tag="is2")
                    nc.vector.tensor_tensor(
                        out=is2[:], in0=l2[:], in1=bcast(m2[:]),
                        op=OP.is_equal
                    )
                    # s1 = sigmoid(m1-m2) on ACT; s2 = 1-s1 via sigmoid(-d)
                    d12 = r_pool.tile([P, NTC], F32, name=f"d12{it}",
                                      tag="d12")
                    nc.vector.tensor_sub(out=d12[:], in0=m1[:], in1=m2[:])
                    s1 = r_pool.tile([P, NTC], F32, name=f"s1{it}", tag="s1")
                    nc.scalar.activation(
                        s1[:], d12[:], mybir.ActivationFunctionType.Sigmoid)
                    s2 = r_pool.tile([P, NTC], F32, name=f"s2{it}", tag="s2")
                    nc.scalar.activation(
                        s2[:], d12[:], mybir.ActivationFunctionType.Sigmoid,
                        scale=-1.0
                    )
                    cw = r_pool.tile([P, NTC, E], F32, name=f"cw{it}",
                                     tag="cw")
                    nc.vector.tensor_tensor(
                        out=cw[:], in0=is1[:], in1=bcast(s1[:]), op=OP.mult
                    )
                    cw2 = r_pool.tile([P, NTC, E], F32, name=f"cw2{it}",
                                      tag="cw2")
                    nc.vector.tensor_tensor(
                        out=cw2[:], in0=is2[:], in1=bcast(s2[:]), op=OP.mult
                    )
                    nc.vector.tensor_add(out=cw[:], in0=cw[:], in1=cw2[:])

                    # tw[t, (e r)] = t_down[t, (e r)] * cw[t, e]; transpose
                    # to [er, t] for the up-projection stationary operand.
                    # i-order matches q1's bank order so pb6/7 free first.
                    for i in IORD:
                        ts = slice(i * P, (i + 1) * P)
                        tw_sb = r_pool.tile([P, ER], F16, name=f"tw_sb{i}_{it}",
                                            tag="tw_sb")
                        nc.vector.tensor_tensor(
                            out=tw_sb[:].rearrange("p (e r) -> p e r", r=R),
                            in0=ga_tiles[i][:, 0:ER].rearrange(
                                "p (e r) -> p e r", r=R),
                            in1=cw[:, i, :].rearrange(
                                "p e -> p e ()").broadcast_to([P, E, R]),
                            op=OP.mult,
                        )
                        twT_ps = pp.tile([P, P], F16, name=f"twT_ps{i}_{it}",
                                         tag=f"pb{4 + i}")
                        nc.tensor.transpose(twT_ps[:], tw_sb[:], identity[:])
                        nc.vector.tensor_copy(twT_sb[:, ts], twT_ps[:])

                # ---- per-iteration program ----
                ga_tiles = [
                    pp.tile([P, GA], F32, name=f"ga_ps{i}_{it}",
                            tag=f"pb{4 + i}")
                    for i in range(NTC)
                ]
                accs0 = quarter0(ga_tiles)
                router_math(ga_tiles)              # DVE/ACT; frees pb4-7
                accs1 = base_quarter(1)            # pb4-7, k-major, up-first
                up_close(0, accs0)                 # twT ready by now
                evict(0, accs0)
                accs2 = base_quarter(2)            # pb0-3 after q0 evict
                evict(1, accs1)
                base_quarter(3, tc_major=True, evict_per_tc=True)  # pb4-7
                evict(2, accs2)

            for it in range(unroll):
                body(it)

    nc.compile()
    return nc


_NC_CACHE = {}


def _get_nc(unroll=1):
    if unroll not in _NC_CACHE:
        _NC_CACHE[unroll] = _build_nc(unroll)
    return _NC_CACHE[unroll]


def _prep_in_maps(x, weight, gate_w, A_w, B_w):
    """Host-side fp16 cast + partition-major pre-tiling (all DMAs land
    contiguous per partition)."""
    xf = np.asarray(x, np.float32).reshape(TOKENS, H)
    wT = np.asarray(weight, np.float32).T.astype(np.float16)        # [H, O]
    whq = np.ascontiguousarray(
        wT.reshape(KT, P, NQ, OQ).transpose(2, 1, 0, 3)
    )                                                               # [q,p,k,o]
    acatT = np.asarray(A_w, np.float32).transpose(2, 0, 1).reshape(H, ER)
    gacatT = np.concatenate(
        [acatT, np.asarray(gate_w, np.float32).T], axis=1
    ).astype(np.float16)                                            # [H, GA]
    gah = np.ascontiguousarray(gacatT.reshape(KT, P, GA).transpose(1, 0, 2))
    bcat = np.ascontiguousarray(
        (np.asarray(B_w, np.float32).transpose(0, 2, 1).reshape(ER, O)
         * LORA_ALPHA).astype(np.float16)
    )
    shared = {"whq": whq, "gah": gah, "bcat": bcat}
    in_maps = []
    for c in range(NCORES):
        xTc = xf[c * T : (c + 1) * T, :].T.astype(np.float16)       # [H, T]
        xhc = np.ascontiguousarray(xTc.reshape(KT, P, T).transpose(1, 0, 2))
        in_maps.append({"xh": xhc, **shared})
    return in_maps


def _unpack_out(res):
    outs = []
    for c in range(NCORES):
        arr = res.results[c]["out"]                  # [q, p, tc, oq] fp16
        full = arr.transpose(2, 1, 0, 3).reshape(T, O)
        outs.append(full.astype(np.float32))
    return np.concatenate(outs, axis=0).reshape(B, S, O)


def kernel(x, weight, gate_w, A_w, B_w, _trace=False, **_ignored):
    in_maps = _prep_in_maps(x, weight, gate_w, A_w, B_w)
    nc = _get_nc()
    res = bass_utils.run_bass_kernel_spmd(
        nc, in_maps, core_ids=list(range(NCORES)), trace=_trace
    )
    full = _unpack_out(res)
    if _trace:
        kernel.last_result = res
    return full


# revision 29
# speedup vs baseline: 1.3606x; 1.3606x over previous
"""MoLoRA linear kernel for Trainium2 (8 NeuronCores, SPMD data-parallel).

Computes: out = x @ W.T + alpha * (per-token top-2 routed LoRA)
Sharding: tokens (B*S = 4096) split 8 ways; all weights replicated.

Numerics: single-pass fp16 everywhere (inputs rounded to fp16, fp32 PSUM
accumulation, fp16 output writeback). Measured against an fp64 reference
this gives ~3.2e-4 rel-RMS error (tolerance 2e-2): fp16 rounding of the
operands dominates; top-2 expert selection flips on ~1/4096 tokens with
negligible output effect. Renormalized top-2 softmax == sigmoid of the
top-2 logit gap.

Layout: every DMA is contiguous per partition (host pre-tiles all operands
into [partition, ...] blocks). The base matmul streams W through the PE
(x chunks stationary); the LoRA-up matmul opens or closes each PSUM
accumulation group so its result lands in the same banks for free.

Self-contained: needs numpy + the concourse (bass) stack importable
(falls back to /opt/trn_rl_repo).
"""

import sys

import numpy as np

try:
    import concourse.bass as bass  # noqa: F401
except Exception:  # pragma: no cover
    sys.path.insert(0, "/opt/trn_rl_repo")

import concourse.bacc as bacc
import concourse.mybir as mybir
import concourse.tile as tile
from concourse import bass_utils
from concourse.masks import make_identity

F32 = mybir.dt.float32
F16 = mybir.dt.float16
AX = mybir.AxisListType.X
OP = mybir.AluOpType

# Problem shapes (hardcoded per contract)
B, S, H, O, E, R = 2, 2048, 2048, 2048, 8, 16
ER = E * R            # 128 = stacked lora rank dim, exactly one partition dim
GA = ER + E           # 136 = lora-A cols + gate cols, fused moving operand
TOKENS = B * S        # 4096
NCORES = 8
T = TOKENS // NCORES  # 512 tokens per core
P = 128
KT = H // P           # 16 contraction chunks
NQ = 4                # output quarters
OQ = O // NQ          # 512 cols per quarter = one PSUM bank
NTC = T // P          # 4 token chunks of 128
KC = 5                # ga matmuls trail base by this many k-chunks
NWARM = 26            # PE warmup transposes (ramp the clock during DMA wait)
XH_CHUNKS = [(i, i + 2) for i in range(0, 16, 2)]
GAH_POS = 4           # gah DMA goes after this many xh chunks
WQ0_CHUNKS = [(i, i + 2) for i in range(0, 16, 2)]
LORA_ALPHA = 16.0
NEG_BIG = 1.0e30
IORD = (2, 3, 0, 1)   # token-chunk processing order (router frees pb6/7 first)


def _build_nc(unroll=1):
    """Build the per-core bass program (identical on all 8 cores).

    unroll>1 repeats the whole computation in one NEFF (same inputs,
    same outputs) — used only for steady-state timing measurements."""
    nc = bacc.Bacc(None, target_bir_lowering=False, debug=False)

    xh = nc.dram_tensor("xh", [P, KT, T], F16, kind="ExternalInput")
    whq = nc.dram_tensor("whq", [NQ, P, KT, OQ], F16, kind="ExternalInput")
    gah = nc.dram_tensor("gah", [P, KT, GA], F16, kind="ExternalInput")
    bcat = nc.dram_tensor("bcat", [ER, O], F16, kind="ExternalInput")
    out = nc.dram_tensor("out", [NQ, P, NTC, OQ], F16, kind="ExternalOutput")

    whq_r = whq[:, :, :, :]

    with tile.TileContext(nc) as tc:
        with (
            tc.tile_pool(name="const", bufs=1) as const_pool,
            tc.tile_pool(name="big", bufs=1) as big_pool,
            tc.tile_pool(name="wstream", bufs=2) as w_pool,
            tc.tile_pool(name="ostage", bufs=2) as o_pool,
            tc.tile_pool(name="router", bufs=1) as r_pool,
            tc.tile_pool(name="psum", bufs=1, space="PSUM") as pp,
        ):
            identity = const_pool.tile([P, P], F16)
            make_identity(nc, identity)

            def body(it):
                if it == 0:
                    # PE warmup: near-dependency-free transposes spin the PE
                    # from t~0.2us so the clock is fully ramped when the
                    # first real matmul's DMA prerequisites land. Sub-writes
                    # into one PSUM tile (no WAW) keep the busy streak
                    # continuous; the junk operand is a 1-op memset (faster
                    # to produce than make_identity's chain).
                    junk = const_pool.tile([P, P], F16, name="junk")
                    nc.vector.memset(junk[:], 0.0)
                    wtile = pp.tile([P, 4, P], F32, name="warm", tag="pb0")
                    for j in range(NWARM):
                        nc.tensor.matmul(wtile[:, j % 4, :], lhsT=junk[:],
                                         rhs=junk[:], start=True, stop=True)
                # ---- resident loads (ACT ring: x + gah; SP: W + bcat) ----
                xh_sb = big_pool.tile([P, KT, T], F16, name=f"xh_sb{it}",
                                      tag="xh_sb")
                gah_sb = big_pool.tile([P, KT, GA], F16, name=f"gah_sb{it}",
                                       tag="gah_sb")
                for ci, (lo, hi) in enumerate(XH_CHUNKS):
                    if ci == GAH_POS:
                        nc.scalar.dma_start(out=gah_sb[:], in_=gah[:, :, :])
                    nc.scalar.dma_start(out=xh_sb[:, lo:hi, :],
                                        in_=xh[:, lo:hi, :])
                if GAH_POS >= len(XH_CHUNKS):
                    nc.scalar.dma_start(out=gah_sb[:], in_=gah[:, :, :])
                bcat_sb = big_pool.tile([P, O], F16, name=f"bcat_sb{it}",
                                        tag="bcat_sb")

                twT_sb = big_pool.tile([P, T], F16, name=f"twT_sb{it}",
                                       tag="twT_sb")  # weighted lora-down

                def wq_load(q):
                    """Stream quarter q's weights into a resident tile."""
                    wq = w_pool.tile([P, KT, OQ], F16, name=f"wq{q}_{it}",
                                     tag="wq")
                    if q == 0:
                        chunks = WQ0_CHUNKS
                    elif q == 1:
                        chunks = [(0, 2), (2, 4), (4, 8), (8, 16)]
                    else:
                        chunks = [(0, 8), (8, 16)]
                    for lo, hi in chunks:
                        nc.sync.dma_start(out=wq[:, lo:hi, :],
                                          in_=whq_r[q, :, lo:hi, :])
                    if q == 1:
                        # bcat rides the W ring; only needed by up_close(0)
                        nc.sync.dma_start(out=bcat_sb[:], in_=bcat[:, :])
                    return wq

                def quarter0(ga_tiles):
                    """O-quarter 0 (banks pb0-3) + the lora-down/logits
                    matmuls (pb4-7, trailing by KC so the gate tensor never
                    blocks the in-order PE queue during DMA rampup)."""
                    wq = wq_load(0)
                    accs = [
                        pp.tile([P, OQ], F32, name=f"acc0_{i}_{it}",
                                tag=f"pb{i}")
                        for i in range(NTC)
                    ]

                    def ga_mms(k):
                        for i in IORD:
                            ts = slice(i * P, (i + 1) * P)
                            nc.tensor.matmul(
                                ga_tiles[i][:], lhsT=xh_sb[:, k, ts],
                                rhs=gah_sb[:, k, :], start=(k == 0),
                                stop=(k == KT - 1),
                            )

                    for k in range(KT):
                        for i in range(NTC):
                            ts = slice(i * P, (i + 1) * P)
                            nc.tensor.matmul(
                                accs[i][:], lhsT=xh_sb[:, k, ts],
                                rhs=wq[:, k, :], start=(k == 0), stop=False,
                            )
                        if k >= KC:
                            ga_mms(k - KC)
                    for k in range(KT - KC, KT):
                        ga_mms(k)
                    return accs

                def base_quarter(q, up_first, close, tc_major=False,
                                 evict_per_tc=False):
                    """One O-quarter of the base matmul; banks alternate
                    between pb0-3 (even q) and pb4-7 (odd q). If up_first,
                    the lora up-projection opens each accumulation group;
                    if not close, the group is left open for up_close."""
                    cols = slice(q * OQ, (q + 1) * OQ)
                    bank = (q % 2) * 4
                    wq = wq_load(q)
                    accs = {}
                    order = IORD if (tc_major or q == 1) else range(NTC)

                    def open_acc(i):
                        accs[i] = pp.tile([P, OQ], F32,
                                          name=f"acc{q}_{i}_{it}",
                                          tag=f"pb{bank + i}")
                        if up_first:
                            ts = slice(i * P, (i + 1) * P)
                            nc.tensor.matmul(
                                accs[i][:], lhsT=twT_sb[:, ts],
                                rhs=bcat_sb[:, cols], start=True, stop=False,
                            )

                    if tc_major:
                        for i in order:
                            ts = slice(i * P, (i + 1) * P)
                            open_acc(i)
                            for k in range(KT):
                                nc.tensor.matmul(
                                    accs[i][:], lhsT=xh_sb[:, k, ts],
                                    rhs=wq[:, k, :],
                                    start=(k == 0 and not up_first),
                                    stop=(close and k == KT - 1),
                                )
                            if evict_per_tc:
                                evict_tc(q, i, accs[i])
                    else:
                        for i in order:
                            open_acc(i)
                        for k in range(KT):
                            for i in order:
                                ts = slice(i * P, (i + 1) * P)
                                nc.tensor.matmul(
                                    accs[i][:], lhsT=xh_sb[:, k, ts],
                                    rhs=wq[:, k, :],
                                    start=(k == 0 and not up_first),
                                    stop=(close and k == KT - 1),
                                )
                    return accs

                def up_close(q, accs):
                    """Close each accumulation group with the up matmul."""
                    for i in range(NTC):
                        ts = slice(i * P, (i + 1) * P)
                        nc.tensor.matmul(
                            accs[i][:], lhsT=twT_sb[:, ts],
                            rhs=bcat_sb[:, q * OQ : (q + 1) * OQ],
                            start=False, stop=True,
                        )

                def evict(q, accs):
                    """Cast accs to fp16, ship the quarter in one DMA."""
                    ost = o_pool.tile([P, NTC, OQ], F16, name=f"ost{q}_{it}",
                                      tag="ost")
                    for i in range(NTC):
                        nc.vector.tensor_copy(ost[:, i, :], accs[i][:])
                    nc.scalar.dma_start(out=out[q, :, :, :], in_=ost[:])

                def evict_tc(q, i, acc, halves=False):
                    ost = o_pool.tile([P, OQ], F16, name=f"ost{q}_{i}_{it}",
                                      tag="ostc")
                    if halves:
                        # uneven split so the very last DMA is tiny
                        for hs in (slice(0, 384), slice(384, OQ)):
                            nc.vector.tensor_copy(ost[:, hs], acc[:, hs])
                            nc.scalar.dma_start(out=out[q, :, i, hs],
                                                in_=ost[:, hs])
                    else:
                        nc.vector.tensor_copy(ost[:], acc[:])
                        nc.scalar.dma_start(out=out[q, :, i, :], in_=ost[:])

                def router_math(tdown_tiles):
                    """Per-token-chunk top-2 routing from the SBUF copies
                    of the lora-down/logits block. Runs entirely on DVE/ACT
                    underneath quarter 1's matmuls; only the up_close /
                    up-first matmuls later depend on its twT output."""
                    for i in IORD:
                        ts = slice(i * P, (i + 1) * P)
                        g = tdown_tiles[i]
                        l = r_pool.tile([P, E], F32, name=f"l{i}_{it}",
                                        tag=f"l{i}")
                        nc.vector.tensor_copy(l[:], g[:, ER:GA])
                        m1 = r_pool.tile([P, 1], F32, name=f"m1_{i}_{it}",
                                         tag=f"m1_{i}")
                        nc.vector.reduce_max(out=m1[:], in_=l[:], axis=AX)

                        def bc(ap):  # [P, 1] -> [P, E]
                            return ap.broadcast_to([P, E])

                        is1 = r_pool.tile([P, E], F32, name=f"is1_{i}_{it}",
                                          tag=f"is1_{i}")
                        nc.vector.tensor_tensor(
                            out=is1[:], in0=l[:], in1=bc(m1[:]),
                            op=OP.is_equal)
                        l2 = r_pool.tile([P, E], F32, name=f"l2_{i}_{it}",
                                         tag=f"l2_{i}")
                        nc.vector.tensor_scalar(
                            out=l2[:], in0=is1[:], scalar1=-NEG_BIG,
                            scalar2=None, op0=OP.mult)
                        nc.vector.tensor_add(out=l2[:], in0=l2[:], in1=l[:])
                        m2 = r_pool.tile([P, 1], F32, name=f"m2_{i}_{it}",
                                         tag=f"m2_{i}")
                        nc.vector.reduce_max(out=m2[:], in_=l2[:], axis=AX)
                        is2 = r_pool.tile([P, E], F32, name=f"is2_{i}_{it}",
                                          tag=f"is2_{i}")
                        nc.vector.tensor_tensor(
                            out=is2[:], in0=l2[:], in1=bc(m2[:]),
                            op=OP.is_equal)
                        # s1 = sigmoid(m1-m2) on ACT; s2 = 1-s1 = sigmoid(-d)
                        d12 = r_pool.tile([P, 1], F32, name=f"d12_{i}_{it}",
                                          tag=f"d12_{i}")
                        nc.vector.tensor_sub(out=d12[:], in0=m1[:], in1=m2[:])
                        s1 = r_pool.tile([P, 1], F32, name=f"s1_{i}_{it}",
                                         tag=f"s1_{i}")
                        nc.scalar.activation(
                            s1[:], d12[:],
                            mybir.ActivationFunctionType.Sigmoid)
                        s2 = r_pool.tile([P, 1], F32, name=f"s2_{i}_{it}",
                                         tag=f"s2_{i}")
                        nc.scalar.activation(
                            s2[:], d12[:],
                            mybir.ActivationFunctionType.Sigmoid, scale=-1.0)
                        cw = r_pool.tile([P, E], F32, name=f"cw_{i}_{it}",
                                         tag=f"cw_{i}")
                        nc.vector.tensor_tensor(
                            out=cw[:], in0=is1[:], in1=bc(s1[:]), op=OP.mult)
                        cw2 = r_pool.tile([P, E], F32, name=f"cw2_{i}_{it}",
                                          tag=f"cw2_{i}")
                        nc.vector.tensor_tensor(
                            out=cw2[:], in0=is2[:], in1=bc(s2[:]), op=OP.mult)
                        nc.vector.tensor_add(out=cw[:], in0=cw[:], in1=cw2[:])

                        # tw[t, (e r)] = t_down[t, (e r)] * cw[t, e], then
                        # DVE-transpose to [er, t] (up-projection stationary)
                        tw_sb = r_pool.tile([P, ER], F16,
                                            name=f"tw_sb{i}_{it}",
                                            tag=f"tw_sb{i}")
                        nc.vector.tensor_tensor(
                            out=tw_sb[:].rearrange("p (e r) -> p e r", r=R),
                            in0=g[:, 0:ER].rearrange("p (e r) -> p e r", r=R),
                            in1=cw[:].rearrange("p e -> p e ()").broadcast_to(
                                [P, E, R]),
                            op=OP.mult,
                        )
                        nc.scalar.dma_start_transpose(twT_sb[:, ts], tw_sb[:])

                # ---- per-iteration program ----
                ga_tiles = [
                    pp.tile([P, GA], F32, name=f"ga_ps{i}_{it}",
                            tag=f"pb{4 + i}")
                    for i in range(NTC)
                ]
                accs0 = quarter0(ga_tiles)         # pb0-3 open; ga stops k15
                # free pb4-7 fast: one f16 copy each (IORD so q1's first
                # bank releases earliest); router reads these SBUF copies
                tdown = {}
                for i in IORD:
                    td = r_pool.tile([P, GA], F16, name=f"td{i}_{it}",
                                     tag=f"td{i}")
                    nc.vector.tensor_copy(td[:], ga_tiles[i][:])
                    tdown[i] = td
                accs1 = base_quarter(1, up_first=False, close=False)  # pb4-7
                router_math(tdown)                 # DVE/ACT, under q1's PE
                up_close(0, accs0)                 # PE: after q1's k-loop
                up_close(1, accs1)
                evict(0, accs0)
                evict(1, accs1)
                accs2 = base_quarter(2, up_first=True, close=True)   # pb0-3
                evict(2, accs2)                    # DMA overlaps q3's PE
                base_quarter(3, up_first=True, close=True,
                             tc_major=True, evict_per_tc=True)       # pb4-7

            for it in range(unroll):
                body(it)

    nc.compile()
    return nc


_NC_CACHE = {}


def _get_nc(unroll=1):
    if unroll not in _NC_CACHE:
        _NC_CACHE[unroll] = _build_nc(unroll)
    return _NC_CACHE[unroll]


def _prep_in_maps(x, weight, gate_w, A_w, B_w):
    """Host-side fp16 cast + partition-major pre-tiling (all DMAs land
    contiguous per partition)."""
    xf = np.asarray(x, np.float32).reshape(TOKENS, H)
    wT = np.asarray(weight, np.float32).T.astype(np.float16)        # [H, O]
    whq = np.ascontiguousarray(
        wT.reshape(KT, P, NQ, OQ).transpose(2, 1, 0, 3)
    )                                                               # [q,p,k,o]
    acatT = np.asarray(A_w, np.float32).transpose(2, 0, 1).reshape(H, ER)
    gacatT = np.concatenate(
        [acatT, np.asarray(gate_w, np.float32).T], axis=1
    ).astype(np.float16)                                            # [H, GA]
    gah = np.ascontiguousarray(gacatT.reshape(KT, P, GA).transpose(1, 0, 2))
    bcat = np.ascontiguousarray(
        (np.asarray(B_w, np.float32).transpose(0, 2, 1).reshape(ER, O)
         * LORA_ALPHA).astype(np.float16)
    )
    shared = {"whq": whq, "gah": gah, "bcat": bcat}
    in_maps = []
    for c in range(NCORES):
        xTc = xf[c * T : (c + 1) * T, :].T.astype(np.float16)       # [H, T]
        xhc = np.ascontiguousarray(xTc.reshape(KT, P, T).transpose(1, 0, 2))
        in_maps.append({"xh": xhc, **shared})
    return in_maps


def _unpack_out(res):
    outs = []
    for c in range(NCORES):
        arr = res.results[c]["out"]                  # [q, p, tc, oq] fp16
        full = arr.transpose(2, 1, 0, 3).reshape(T, O)
        outs.append(full.astype(np.float32))
    return np.concatenate(outs, axis=0).reshape(B, S, O)


def kernel(x, weight, gate_w, A_w, B_w, _trace=False, **_ignored):
    in_maps = _prep_in_maps(x, weight, gate_w, A_w, B_w)
    nc = _get_nc()
    res = bass_utils.run_bass_kernel_spmd(
        nc, in_maps, core_ids=list(range(NCORES)), trace=_trace
    )
    full = _unpack_out(res)
    if _trace:
        kernel.last_result = res
    return full


# revision 30
# speedup vs baseline: 2.1024x; 1.5452x over previous
"""MoLoRA linear kernel for Trainium2 (8 NeuronCores, SPMD data-parallel).

Computes: out = x @ W.T + alpha * (per-token top-2 routed LoRA)
Sharding: tokens (B*S = 4096) split 8 ways; all weights replicated.

Numerics: single-pass fp16 everywhere (inputs rounded to fp16, fp32 PSUM
accumulation, fp16 output writeback). Measured against an fp64 reference
this gives ~3.2e-4 rel-RMS error (tolerance 2e-2): fp16 rounding of the
operands dominates; top-2 expert selection flips on ~1/4096 tokens with
negligible output effect. Renormalized top-2 softmax == sigmoid of the
top-2 logit gap.

Layout: every DMA is contiguous per partition (host pre-tiles all operands
into [partition, ...] blocks). The base matmul streams W through the PE
(x chunks stationary); the LoRA-up matmul opens or closes each PSUM
accumulation group so its result lands in the same banks for free.

Self-contained: needs numpy + the concourse (bass) stack importable
(falls back to /opt/trn_rl_repo).
"""

import sys

import numpy as np

try:
    import concourse.bass as bass  # noqa: F401
except Exception:  # pragma: no cover
    sys.path.insert(0, "/opt/trn_rl_repo")

import concourse.bacc as bacc
import concourse.mybir as mybir
import concourse.tile as tile
from concourse import bass_utils

F32 = mybir.dt.float32
F16 = mybir.dt.float16
AX = mybir.AxisListType.X
OP = mybir.AluOpType

# Problem shapes (hardcoded per contract)
B, S, H, O, E, R = 2, 2048, 2048, 2048, 8, 16
ER = E * R            # 128 = stacked lora rank dim, exactly one partition dim
GA = ER + E           # 136 = lora-A cols + gate cols, fused moving operand
TOKENS = B * S        # 4096
NCORES = 8
T = TOKENS // NCORES  # 512 tokens per core
P = 128
KT = H // P           # 16 contraction chunks
NQ = 4                # output quarters
OQ = O // NQ          # 512 cols per quarter = one PSUM bank
NTC = T // P          # 4 token chunks of 128
KC = 5                # ga matmuls trail base by this many k-chunks
NWARM = 26            # PE warmup transposes (ramp the clock during DMA wait)
XH_CHUNKS = [(i, i + 2) for i in range(0, 16, 2)]
GAH_POS = 4           # gah DMA goes after this many xh chunks
WQ0_CHUNKS = [(i, i + 2) for i in range(0, 16, 2)]
LORA_ALPHA = 16.0
NEG_BIG = 1.0e30
IORD = (2, 3, 0, 1)   # token-chunk processing order (router frees pb6/7 first)


def _build_nc(unroll=1):
    """Build the per-core bass program (identical on all 8 cores).

    unroll>1 repeats the whole computation in one NEFF (same inputs,
    same outputs) — used only for steady-state timing measurements."""
    nc = bacc.Bacc(None, target_bir_lowering=False, debug=False)

    xh = nc.dram_tensor("xh", [P, KT, T], F16, kind="ExternalInput")
    whq = nc.dram_tensor("whq", [NQ, P, KT, OQ], F16, kind="ExternalInput")
    gah = nc.dram_tensor("gah", [P, KT, GA], F16, kind="ExternalInput")
    bcat = nc.dram_tensor("bcat", [ER, O], F16, kind="ExternalInput")
    out = nc.dram_tensor("out", [NQ, P, NTC, OQ], F16, kind="ExternalOutput")

    whq_r = whq[:, :, :, :]

    with tile.TileContext(nc) as tc:
        with (
            tc.tile_pool(name="const", bufs=1) as const_pool,
            tc.tile_pool(name="big", bufs=1) as big_pool,
            tc.tile_pool(name="wstream", bufs=2) as w_pool,
            tc.tile_pool(name="ostage", bufs=2) as o_pool,
            tc.tile_pool(name="router", bufs=1) as r_pool,
            tc.tile_pool(name="psum", bufs=1, space="PSUM") as pp,
        ):
            def body(it):
                if it == 0:
                    # PE warmup: near-dependency-free transposes spin the PE
                    # from t~0.2us so the clock is fully ramped when the
                    # first real matmul's DMA prerequisites land. Sub-writes
                    # into one PSUM tile (no WAW) keep the busy streak
                    # continuous; the junk operand is a 1-op memset (faster
                    # to produce than make_identity's chain).
                    junk = const_pool.tile([P, P], F16, name="junk")
                    nc.vector.memset(junk[:], 0.0)
                    wtile = pp.tile([P, 4, P], F32, name="warm", tag="pb0")
                    for j in range(NWARM):
                        nc.tensor.matmul(wtile[:, j % 4, :], lhsT=junk[:],
                                         rhs=junk[:], start=True, stop=True)
                # ---- resident loads (ACT ring: x + gah; SP: W + bcat) ----
                xh_sb = big_pool.tile([P, KT, T], F16, name=f"xh_sb{it}",
                                      tag="xh_sb")
                gah_sb = big_pool.tile([P, KT, GA], F16, name=f"gah_sb{it}",
                                       tag="gah_sb")
                for ci, (lo, hi) in enumerate(XH_CHUNKS):
                    if ci == GAH_POS:
                        nc.scalar.dma_start(out=gah_sb[:], in_=gah[:, :, :])
                    nc.scalar.dma_start(out=xh_sb[:, lo:hi, :],
                                        in_=xh[:, lo:hi, :])
                if GAH_POS >= len(XH_CHUNKS):
                    nc.scalar.dma_start(out=gah_sb[:], in_=gah[:, :, :])
                bcat_sb = big_pool.tile([P, O], F16, name=f"bcat_sb{it}",
                                        tag="bcat_sb")

                twT_sb = big_pool.tile([P, T], F16, name=f"twT_sb{it}",
                                       tag="twT_sb")  # weighted lora-down

                def wq_load(q):
                    """Stream quarter q's weights into a resident tile."""
                    wq = w_pool.tile([P, KT, OQ], F16, name=f"wq{q}_{it}",
                                     tag="wq")
                    if q == 0:
                        chunks = WQ0_CHUNKS
                    elif q == 1:
                        chunks = [(0, 2), (2, 4), (4, 8), (8, 16)]
                    else:
                        chunks = [(0, 8), (8, 16)]
                    for lo, hi in chunks:
                        nc.sync.dma_start(out=wq[:, lo:hi, :],
                                          in_=whq_r[q, :, lo:hi, :])
                    if q == 1:
                        # bcat rides the W ring; only needed by up_close(0)
                        nc.sync.dma_start(out=bcat_sb[:], in_=bcat[:, :])
                    return wq

                def quarter0(ga_tiles):
                    """O-quarter 0 (banks pb0-3) + the lora-down/logits
                    matmuls (pb4-7, trailing by KC so the gate tensor never
                    blocks the in-order PE queue during DMA rampup)."""
                    wq = wq_load(0)
                    accs = [
                        pp.tile([P, OQ], F32, name=f"acc0_{i}_{it}",
                                tag=f"pb{i}")
                        for i in range(NTC)
                    ]

                    def ga_mms(k):
                        for i in IORD:
                            ts = slice(i * P, (i + 1) * P)
                            nc.tensor.matmul(
                                ga_tiles[i][:], lhsT=xh_sb[:, k, ts],
                                rhs=gah_sb[:, k, :], start=(k == 0),
                                stop=(k == KT - 1),
                            )

                    for k in range(KT):
                        for i in range(NTC):
                            ts = slice(i * P, (i + 1) * P)
                            nc.tensor.matmul(
                                accs[i][:], lhsT=xh_sb[:, k, ts],
                                rhs=wq[:, k, :], start=(k == 0), stop=False,
                            )
                        if k >= KC:
                            ga_mms(k - KC)
                    for k in range(KT - KC, KT):
                        ga_mms(k)
                    return accs

                def base_quarter(q, up_first, close, tc_major=False,
                                 evict_per_tc=False):
                    """One O-quarter of the base matmul; banks alternate
                    between pb0-3 (even q) and pb4-7 (odd q). If up_first,
                    the lora up-projection opens each accumulation group;
                    if not close, the group is left open for up_close."""
                    cols = slice(q * OQ, (q + 1) * OQ)
                    bank = (q % 2) * 4
                    wq = wq_load(q)
                    accs = {}
                    order = IORD if (tc_major or q == 1) else range(NTC)

                    def open_acc(i):
                        accs[i] = pp.tile([P, OQ], F32,
                                          name=f"acc{q}_{i}_{it}",
                                          tag=f"pb{bank + i}")
                        if up_first:
                            ts = slice(i * P, (i + 1) * P)
                            nc.tensor.matmul(
                                accs[i][:], lhsT=twT_sb[:, ts],
                                rhs=bcat_sb[:, cols], start=True, stop=False,
                            )

                    if tc_major:
                        for i in order:
                            ts = slice(i * P, (i + 1) * P)
                            open_acc(i)
                            for k in range(KT):
                                nc.tensor.matmul(
                                    accs[i][:], lhsT=xh_sb[:, k, ts],
                                    rhs=wq[:, k, :],
                                    start=(k == 0 and not up_first),
                                    stop=(close and k == KT - 1),
                                )
                            if evict_per_tc:
                                evict_tc(q, i, accs[i])
                    else:
                        for i in order:
                            open_acc(i)
                        for k in range(KT):
                            for i in order:
                                ts = slice(i * P, (i + 1) * P)
                                nc.tensor.matmul(
                                    accs[i][:], lhsT=xh_sb[:, k, ts],
                                    rhs=wq[:, k, :],
                                    start=(k == 0 and not up_first),
                                    stop=(close and k == KT - 1),
                                )
                    return accs

                def up_close(q, accs):
                    """Close each accumulation group with the up matmul."""
                    for i in range(NTC):
                        ts = slice(i * P, (i + 1) * P)
                        nc.tensor.matmul(
                            accs[i][:], lhsT=twT_sb[:, ts],
                            rhs=bcat_sb[:, q * OQ : (q + 1) * OQ],
                            start=False, stop=True,
                        )

                def evict(q, accs):
                    """Cast accs to fp16, ship the quarter in one DMA."""
                    ost = o_pool.tile([P, NTC, OQ], F16, name=f"ost{q}_{it}",
                                      tag="ost")
                    for i in range(NTC):
                        nc.vector.tensor_copy(ost[:, i, :], accs[i][:])
                    nc.scalar.dma_start(out=out[q, :, :, :], in_=ost[:])

                def evict_tc(q, i, acc, halves=False):
                    ost = o_pool.tile([P, OQ], F16, name=f"ost{q}_{i}_{it}",
                                      tag="ostc")
                    if halves:
                        # uneven split so the very last DMA is tiny
                        for hs in (slice(0, 384), slice(384, OQ)):
                            nc.vector.tensor_copy(ost[:, hs], acc[:, hs])
                            nc.scalar.dma_start(out=out[q, :, i, hs],
                                                in_=ost[:, hs])
                    else:
                        nc.vector.tensor_copy(ost[:], acc[:])
                        nc.scalar.dma_start(out=out[q, :, i, :], in_=ost[:])

                def router_math(tdown_tiles):
                    """Per-token-chunk top-2 routing from the SBUF copies
                    of the lora-down/logits block. Runs entirely on DVE/ACT
                    underneath quarter 1's matmuls; only the up_close /
                    up-first matmuls later depend on its twT output."""
                    for i in IORD:
                        ts = slice(i * P, (i + 1) * P)
                        g = tdown_tiles[i]
                        l = r_pool.tile([P, E], F32, name=f"l{i}_{it}",
                                        tag=f"l{i}")
                        nc.vector.tensor_copy(l[:], g[:, ER:GA])
                        m1 = r_pool.tile([P, 1], F32, name=f"m1_{i}_{it}",
                                         tag=f"m1_{i}")
                        nc.vector.reduce_max(out=m1[:], in_=l[:], axis=AX)

                        def bc(ap):  # [P, 1] -> [P, E]
                            return ap.broadcast_to([P, E])

                        is1 = r_pool.tile([P, E], F32, name=f"is1_{i}_{it}",
                                          tag=f"is1_{i}")
                        nc.vector.tensor_tensor(
                            out=is1[:], in0=l[:], in1=bc(m1[:]),
                            op=OP.is_equal)
                        l2 = r_pool.tile([P, E], F32, name=f"l2_{i}_{it}",
                                         tag=f"l2_{i}")
                        nc.vector.tensor_scalar(
                            out=l2[:], in0=is1[:], scalar1=-NEG_BIG,
                            scalar2=None, op0=OP.mult)
                        nc.vector.tensor_add(out=l2[:], in0=l2[:], in1=l[:])
                        m2 = r_pool.tile([P, 1], F32, name=f"m2_{i}_{it}",
                                         tag=f"m2_{i}")
                        nc.vector.reduce_max(out=m2[:], in_=l2[:], axis=AX)
                        is2 = r_pool.tile([P, E], F32, name=f"is2_{i}_{it}",
                                          tag=f"is2_{i}")
                        nc.vector.tensor_tensor(
                            out=is2[:], in0=l2[:], in1=bc(m2[:]),
                            op=OP.is_equal)
                        # s1 = sigmoid(m1-m2) on ACT; s2 = 1-s1 = sigmoid(-d)
                        d12 = r_pool.tile([P, 1], F32, name=f"d12_{i}_{it}",
                                          tag=f"d12_{i}")
                        nc.vector.tensor_sub(out=d12[:], in0=m1[:], in1=m2[:])
                        s1 = r_pool.tile([P, 1], F32, name=f"s1_{i}_{it}",
                                         tag=f"s1_{i}")
                        nc.scalar.activation(
                            s1[:], d12[:],
                            mybir.ActivationFunctionType.Sigmoid)
                        s2 = r_pool.tile([P, 1], F32, name=f"s2_{i}_{it}",
                                         tag=f"s2_{i}")
                        nc.scalar.activation(
                            s2[:], d12[:],
                            mybir.ActivationFunctionType.Sigmoid, scale=-1.0)
                        cw = r_pool.tile([P, E], F32, name=f"cw_{i}_{it}",
                                         tag=f"cw_{i}")
                        nc.vector.tensor_tensor(
                            out=cw[:], in0=is1[:], in1=bc(s1[:]), op=OP.mult)
                        cw2 = r_pool.tile([P, E], F32, name=f"cw2_{i}_{it}",
                                          tag=f"cw2_{i}")
                        nc.vector.tensor_tensor(
                            out=cw2[:], in0=is2[:], in1=bc(s2[:]), op=OP.mult)
                        nc.vector.tensor_add(out=cw[:], in0=cw[:], in1=cw2[:])

                        # tw[t, (e r)] = t_down[t, (e r)] * cw[t, e], then
                        # DVE-transpose to [er, t] (up-projection stationary)
                        tw_sb = r_pool.tile([P, ER], F16,
                                            name=f"tw_sb{i}_{it}",
                                            tag=f"tw_sb{i}")
                        nc.vector.tensor_tensor(
                            out=tw_sb[:].rearrange("p (e r) -> p e r", r=R),
                            in0=g[:, 0:ER].rearrange("p (e r) -> p e r", r=R),
                            in1=cw[:].rearrange("p e -> p e ()").broadcast_to(
                                [P, E, R]),
                            op=OP.mult,
                        )
                        nc.scalar.dma_start_transpose(twT_sb[:, ts], tw_sb[:])

                # ---- per-iteration program ----
                ga_tiles = [
                    pp.tile([P, GA], F32, name=f"ga_ps{i}_{it}",
                            tag=f"pb{4 + i}")
                    for i in range(NTC)
                ]
                accs0 = quarter0(ga_tiles)         # pb0-3 open; ga stops k15
                # free pb4-7 fast: one f16 copy each (IORD so q1's first
                # bank releases earliest); router reads these SBUF copies
                tdown = {}
                for i in IORD:
                    td = r_pool.tile([P, GA], F16, name=f"td{i}_{it}",
                                     tag=f"td{i}")
                    nc.vector.tensor_copy(td[:], ga_tiles[i][:])
                    tdown[i] = td
                accs1 = base_quarter(1, up_first=False, close=False)  # pb4-7
                router_math(tdown)                 # DVE/ACT, under q1's PE
                up_close(0, accs0)                 # PE: after q1's k-loop
                up_close(1, accs1)
                evict(0, accs0)
                evict(1, accs1)
                accs2 = base_quarter(2, up_first=True, close=True)   # pb0-3
                evict(2, accs2)                    # DMA overlaps q3's PE
                base_quarter(3, up_first=True, close=True,
                             tc_major=True, evict_per_tc=True)       # pb4-7

            for it in range(unroll):
                body(it)

    nc.compile()
    return nc


_NC_CACHE = {}


def _get_nc(unroll=1):
    if unroll not in _NC_CACHE:
        _NC_CACHE[unroll] = _build_nc(unroll)
    return _NC_CACHE[unroll]


def _prep_in_maps(x, weight, gate_w, A_w, B_w):
    """Host-side fp16 cast + partition-major pre-tiling (all DMAs land
    contiguous per partition)."""
    xf = np.asarray(x, np.float32).reshape(TOKENS, H)
    wT = np.asarray(weight, np.float32).T.astype(np.float16)        # [H, O]
    whq = np.ascontiguousarray(
        wT.reshape(KT, P, NQ, OQ).transpose(2, 1, 0, 3)
    )                                                               # [q,p,k,o]
    acatT = np.asarray(A_w, np.float32).transpose(2, 0, 1).reshape(H, ER)
    gacatT = np.concatenate(
        [acatT, np.asarray(gate_w, np.float32).T], axis=1
    ).astype(np.float16)                                            # [H, GA]
    gah = np.ascontiguousarray(gacatT.reshape(KT, P, GA).transpose(1, 0, 2))
    bcat = np.ascontiguousarray(
        (np.asarray(B_w, np.float32).transpose(0, 2, 1).reshape(ER, O)
         * LORA_ALPHA).astype(np.float16)
    )
    shared = {"whq": whq, "gah": gah, "bcat": bcat}
    in_maps = []
    for c in range(NCORES):
        xTc = xf[c * T : (c + 1) * T, :].T.astype(np.float16)       # [H, T]
        xhc = np.ascontiguousarray(xTc.reshape(KT, P, T).transpose(1, 0, 2))
        in_maps.append({"xh": xhc, **shared})
    return in_maps


def _unpack_out(res):
    outs = []
    for c in range(NCORES):
        arr = res.results[c]["out"]                  # [q, p, tc, oq] fp16
        full = arr.transpose(2, 1, 0, 3).reshape(T, O)
        outs.append(full.astype(np.float32))
    return np.concatenate(outs, axis=0).reshape(B, S, O)


def kernel(x, weight, gate_w, A_w, B_w, _trace=False, **_ignored):
    in_maps = _prep_in_maps(x, weight, gate_w, A_w, B_w)
    nc = _get_nc()
    res = bass_utils.run_bass_kernel_spmd(
        nc, in_maps, core_ids=list(range(NCORES)), trace=_trace
    )
    full = _unpack_out(res)
    if _trace:
        kernel.last_result = res
    return full


# revision 32
# speedup vs baseline: 2.4418x; 1.1615x over previous
"""MoLoRA linear kernel for Trainium2 (8 NeuronCores, SPMD data-parallel).

Computes: out = x @ W.T + alpha * (per-token top-2 routed LoRA)
Sharding: tokens (B*S = 4096) split 8 ways; all weights replicated.

Numerics: single-pass fp16 everywhere (inputs rounded to fp16, fp32 PSUM
accumulation, fp16 output writeback). Measured against an fp64 reference
this gives ~3.2e-4 rel-RMS error (tolerance 2e-2): fp16 rounding of the
operands dominates; top-2 expert selection flips on ~1/4096 tokens with
negligible output effect. Renormalized top-2 softmax == sigmoid of the
top-2 logit gap.

Layout: every DMA is contiguous per partition (host pre-tiles all operands
into [partition, ...] blocks). The base matmul streams W through the PE
(x chunks stationary); the LoRA-up matmul opens or closes each PSUM
accumulation group so its result lands in the same banks for free.

Self-contained: needs numpy + the concourse (bass) stack importable
(falls back to /opt/trn_rl_repo).
"""

import sys

import numpy as np

try:
    import concourse.bass as bass  # noqa: F401
except Exception:  # pragma: no cover
    sys.path.insert(0, "/opt/trn_rl_repo")

import concourse.bacc as bacc
import concourse.mybir as mybir
import concourse.tile as tile
from concourse import bass_utils

F32 = mybir.dt.float32
F16 = mybir.dt.float16
AX = mybir.AxisListType.X
OP = mybir.AluOpType

# Problem shapes (hardcoded per contract)
B, S, H, O, E, R = 2, 2048, 2048, 2048, 8, 16
ER = E * R            # 128 = stacked lora rank dim, exactly one partition dim
GA = ER + E           # 136 = lora-A cols + gate cols, fused moving operand
TOKENS = B * S        # 4096
NCORES = 8
T = TOKENS // NCORES  # 512 tokens per core
P = 128
KT = H // P           # 16 contraction chunks
NQ = 4                # output quarters
OQ = O // NQ          # 512 cols per quarter = one PSUM bank
NTC = T // P          # 4 token chunks of 128
KC = 5                # ga matmuls trail base by this many k-chunks
NWARM = 26            # PE warmup transposes (ramp the clock during DMA wait)
XH_CHUNKS = [(i, i + 2) for i in range(0, 16, 2)]
GAH_POS = 4           # gah DMA goes after this many xh chunks
WQ0_CHUNKS = [(i, i + 2) for i in range(0, 16, 2)]
LORA_ALPHA = 16.0
NEG_BIG = 1.0e30
IORD = (2, 3, 0, 1)   # token-chunk processing order (router frees pb6/7 first)


def _build_nc(unroll=1):
    """Build the per-core bass program (identical on all 8 cores).

    unroll>1 repeats the whole computation in one NEFF (same inputs,
    same outputs) — used only for steady-state timing measurements."""
    nc = bacc.Bacc(None, target_bir_lowering=False, debug=False)

    xh = nc.dram_tensor("xh", [P, KT, T], F16, kind="ExternalInput")
    whq = nc.dram_tensor("whq", [NQ, P, KT, OQ], F16, kind="ExternalInput")
    gah = nc.dram_tensor("gah", [P, KT, GA], F16, kind="ExternalInput")
    bcat = nc.dram_tensor("bcat", [ER, O], F16, kind="ExternalInput")
    out = nc.dram_tensor("out", [NQ, P, NTC, OQ], F16, kind="ExternalOutput")

    whq_r = whq[:, :, :, :]

    with tile.TileContext(nc) as tc:
        with (
            tc.tile_pool(name="const", bufs=1) as const_pool,
            tc.tile_pool(name="big", bufs=1) as big_pool,
            tc.tile_pool(name="wstream", bufs=2) as w_pool,
            tc.tile_pool(name="ostage", bufs=2) as o_pool,
            tc.tile_pool(name="router", bufs=1) as r_pool,
            tc.tile_pool(name="psum", bufs=1, space="PSUM") as pp,
        ):
            def body(it):
                if it == 0:
                    # PE warmup: near-dependency-free transposes spin the PE
                    # from t~0.2us so the clock is fully ramped when the
                    # first real matmul's DMA prerequisites land. Sub-writes
                    # into one PSUM tile (no WAW) keep the busy streak
                    # continuous; the junk operand is a 1-op memset (faster
                    # to produce than make_identity's chain).
                    junk = const_pool.tile([P, P], F16, name="junk")
                    nc.vector.memset(junk[:], 0.0)
                    wtile = pp.tile([P, 4, P], F32, name="warm", tag="pb0")
                    for j in range(NWARM):
                        nc.tensor.matmul(wtile[:, j % 4, :], lhsT=junk[:],
                                         rhs=junk[:], start=True, stop=True)
                # ---- resident loads (ACT ring: x + gah; SP: W + bcat) ----
                xh_sb = big_pool.tile([P, KT, T], F16, name=f"xh_sb{it}",
                                      tag="xh_sb")
                gah_sb = big_pool.tile([P, KT, GA], F16, name=f"gah_sb{it}",
                                       tag="gah_sb")
                for ci, (lo, hi) in enumerate(XH_CHUNKS):
                    if ci == GAH_POS:
                        nc.scalar.dma_start(out=gah_sb[:, 0:8, :],
                                            in_=gah[:, 0:8, :])
                    if ci == GAH_POS + 3:
                        nc.scalar.dma_start(out=gah_sb[:, 8:16, :],
                                            in_=gah[:, 8:16, :])
                    nc.scalar.dma_start(out=xh_sb[:, lo:hi, :],
                                        in_=xh[:, lo:hi, :])
                bcat_sb = big_pool.tile([P, O], F16, name=f"bcat_sb{it}",
                                        tag="bcat_sb")

                twT_sb = big_pool.tile([P, T], F16, name=f"twT_sb{it}",
                                       tag="twT_sb")  # weighted lora-down

                def wq_load(q):
                    """Stream quarter q's weights into a resident tile."""
                    wq = w_pool.tile([P, KT, OQ], F16, name=f"wq{q}_{it}",
                                     tag="wq")
                    if q == 0:
                        chunks = WQ0_CHUNKS
                    elif q == 1:
                        chunks = [(0, 2), (2, 4), (4, 8), (8, 16)]
                    else:
                        chunks = [(0, 8), (8, 16)]
                    for lo, hi in chunks:
                        nc.sync.dma_start(out=wq[:, lo:hi, :],
                                          in_=whq_r[q, :, lo:hi, :])
                    if q == 1:
                        # bcat rides the W ring; only needed by up_close(0)
                        nc.sync.dma_start(out=bcat_sb[:], in_=bcat[:, :])
                    return wq

                def quarter0(ga_tiles):
                    """O-quarter 0 (banks pb0-3) + the lora-down/logits
                    matmuls (pb4-7, trailing by KC so the gate tensor never
                    blocks the in-order PE queue during DMA rampup)."""
                    wq = wq_load(0)
                    accs = [
                        pp.tile([P, OQ], F32, name=f"acc0_{i}_{it}",
                                tag=f"pb{i}")
                        for i in range(NTC)
                    ]

                    def ga_mms(k):
                        for i in IORD:
                            ts = slice(i * P, (i + 1) * P)
                            nc.tensor.matmul(
                                ga_tiles[i][:], lhsT=xh_sb[:, k, ts],
                                rhs=gah_sb[:, k, :], start=(k == 0),
                                stop=(k == KT - 1),
                            )

                    for k in range(KT):
                        for i in range(NTC):
                            ts = slice(i * P, (i + 1) * P)
                            nc.tensor.matmul(
                                accs[i][:], lhsT=xh_sb[:, k, ts],
                                rhs=wq[:, k, :], start=(k == 0), stop=False,
                            )
                        if k >= KC:
                            ga_mms(k - KC)
                    for k in range(KT - KC, KT):
                        ga_mms(k)
                    return accs

                def base_quarter(q, up_first, close, tc_major=False,
                                 evict_per_tc=False):
                    """One O-quarter of the base matmul; banks alternate
                    between pb0-3 (even q) and pb4-7 (odd q). If up_first,
                    the lora up-projection opens each accumulation group;
                    if not close, the group is left open for up_close."""
                    cols = slice(q * OQ, (q + 1) * OQ)
                    bank = (q % 2) * 4
                    wq = wq_load(q)
                    accs = {}
                    order = IORD if (tc_major or q == 1) else range(NTC)

                    def open_acc(i):
                        accs[i] = pp.tile([P, OQ], F32,
                                          name=f"acc{q}_{i}_{it}",
                                          tag=f"pb{bank + i}")
                        if up_first:
                            ts = slice(i * P, (i + 1) * P)
                            nc.tensor.matmul(
                                accs[i][:], lhsT=twT_sb[:, ts],
                                rhs=bcat_sb[:, cols], start=True, stop=False,
                            )

                    if tc_major:
                        for i in order:
                            ts = slice(i * P, (i + 1) * P)
                            open_acc(i)
                            for k in range(KT):
                                nc.tensor.matmul(
                                    accs[i][:], lhsT=xh_sb[:, k, ts],
                                    rhs=wq[:, k, :],
                                    start=(k == 0 and not up_first),
                                    stop=(close and k == KT - 1),
                                )
                            if evict_per_tc:
                                evict_tc(q, i, accs[i])
                    else:
                        for i in order:
                            open_acc(i)
                        for k in range(KT):
                            for i in order:
                                ts = slice(i * P, (i + 1) * P)
                                nc.tensor.matmul(
                                    accs[i][:], lhsT=xh_sb[:, k, ts],
                                    rhs=wq[:, k, :],
                                    start=(k == 0 and not up_first),
                                    stop=(close and k == KT - 1),
                                )
                    return accs

                def up_close(q, accs):
                    """Close each accumulation group with the up matmul."""
                    for i in range(NTC):
                        ts = slice(i * P, (i + 1) * P)
                        nc.tensor.matmul(
                            accs[i][:], lhsT=twT_sb[:, ts],
                            rhs=bcat_sb[:, q * OQ : (q + 1) * OQ],
                            start=False, stop=True,
                        )

                def evict(q, accs):
                    """Cast accs to fp16, ship the quarter in one DMA."""
                    ost = o_pool.tile([P, NTC, OQ], F16, name=f"ost{q}_{it}",
                                      tag="ost")
                    for i in range(NTC):
                        nc.vector.tensor_copy(ost[:, i, :], accs[i][:])
                    nc.scalar.dma_start(out=out[q, :, :, :], in_=ost[:])

                def evict_tc(q, i, acc):
                    # SP ring: idle once wq3 is in, so the final-quarter
                    # evictions never queue behind the 2MB quarter DMAs
                    ost = o_pool.tile([P, OQ], F16, name=f"ost{q}_{i}_{it}",
                                      tag="ostc")
                    nc.vector.tensor_copy(ost[:], acc[:])
                    nc.sync.dma_start(out=out[q, :, i, :], in_=ost[:])

                def router_math(tdown_tiles):
                    """Per-token-chunk top-2 routing from the SBUF copies
                    of the lora-down/logits block. Runs entirely on DVE/ACT
                    underneath quarter 1's matmuls; only the up_close /
                    up-first matmuls later depend on its twT output."""
                    for i in IORD:
                        ts = slice(i * P, (i + 1) * P)
                        g = tdown_tiles[i]
                        l = r_pool.tile([P, E], F32, name=f"l{i}_{it}",
                                        tag=f"l{i}")
                        nc.vector.tensor_copy(l[:], g[:, ER:GA])
                        m1 = r_pool.tile([P, 1], F32, name=f"m1_{i}_{it}",
                                         tag=f"m1_{i}")
                        nc.vector.reduce_max(out=m1[:], in_=l[:], axis=AX)

                        def bc(ap):  # [P, 1] -> [P, E]
                            return ap.broadcast_to([P, E])

                        is1 = r_pool.tile([P, E], F32, name=f"is1_{i}_{it}",
                                          tag=f"is1_{i}")
                        nc.vector.tensor_tensor(
                            out=is1[:], in0=l[:], in1=bc(m1[:]),
                            op=OP.is_equal)
                        l2 = r_pool.tile([P, E], F32, name=f"l2_{i}_{it}",
                                         tag=f"l2_{i}")
                        nc.vector.tensor_scalar(
                            out=l2[:], in0=is1[:], scalar1=-NEG_BIG,
                            scalar2=None, op0=OP.mult)
                        nc.vector.tensor_add(out=l2[:], in0=l2[:], in1=l[:])
                        m2 = r_pool.tile([P, 1], F32, name=f"m2_{i}_{it}",
                                         tag=f"m2_{i}")
                        nc.vector.reduce_max(out=m2[:], in_=l2[:], axis=AX)
                        is2 = r_pool.tile([P, E], F32, name=f"is2_{i}_{it}",
                                          tag=f"is2_{i}")
                        nc.vector.tensor_tensor(
                            out=is2[:], in0=l2[:], in1=bc(m2[:]),
                            op=OP.is_equal)
                        # s1 = sigmoid(m1-m2) on ACT; s2 = 1-s1 = sigmoid(-d)
                        d12 = r_pool.tile([P, 1], F32, name=f"d12_{i}_{it}",
                                          tag=f"d12_{i}")
                        nc.vector.tensor_sub(out=d12[:], in0=m1[:], in1=m2[:])
                        s1 = r_pool.tile([P, 1], F32, name=f"s1_{i}_{it}",
                                         tag=f"s1_{i}")
                        nc.scalar.activation(
                            s1[:], d12[:],
                            mybir.ActivationFunctionType.Sigmoid)
                        s2 = r_pool.tile([P, 1], F32, name=f"s2_{i}_{it}",
                                         tag=f"s2_{i}")
                        nc.scalar.activation(
                            s2[:], d12[:],
                            mybir.ActivationFunctionType.Sigmoid, scale=-1.0)
                        cw = r_pool.tile([P, E], F32, name=f"cw_{i}_{it}",
                                         tag=f"cw_{i}")
                        nc.vector.tensor_tensor(
                            out=cw[:], in0=is1[:], in1=bc(s1[:]), op=OP.mult)
                        cw2 = r_pool.tile([P, E], F32, name=f"cw2_{i}_{it}",
                                          tag=f"cw2_{i}")
                        nc.vector.tensor_tensor(
                            out=cw2[:], in0=is2[:], in1=bc(s2[:]), op=OP.mult)
                        nc.vector.tensor_add(out=cw[:], in0=cw[:], in1=cw2[:])

                        # tw[t, (e r)] = t_down[t, (e r)] * cw[t, e], then
                        # DVE-transpose to [er, t] (up-projection stationary)
                        tw_sb = r_pool.tile([P, ER], F16,
                                            name=f"tw_sb{i}_{it}",
                                            tag=f"tw_sb{i}")
                        nc.vector.tensor_tensor(
                            out=tw_sb[:].rearrange("p (e r) -> p e r", r=R),
                            in0=g[:, 0:ER].rearrange("p (e r) -> p e r", r=R),
                            in1=cw[:].rearrange("p e -> p e ()").broadcast_to(
                                [P, E, R]),
                            op=OP.mult,
                        )
                        nc.scalar.dma_start_transpose(twT_sb[:, ts], tw_sb[:])

                # ---- per-iteration program ----
                ga_tiles = [
                    pp.tile([P, GA], F32, name=f"ga_ps{i}_{it}",
                            tag=f"pb{4 + i}")
                    for i in range(NTC)
                ]
                accs0 = quarter0(ga_tiles)         # pb0-3 open; ga stops k15
                # free pb4-7 fast: one f16 copy each (IORD so q1's first
                # bank releases earliest); router reads these SBUF copies
                tdown = {}
                for i in IORD:
                    td = r_pool.tile([P, GA], F16, name=f"td{i}_{it}",
                                     tag=f"td{i}")
                    nc.vector.tensor_copy(td[:], ga_tiles[i][:])
                    tdown[i] = td
                accs1 = base_quarter(1, up_first=False, close=False)  # pb4-7
                router_math(tdown)                 # DVE/ACT, under q1's PE
                up_close(0, accs0)                 # PE: after q1's k-loop
                up_close(1, accs1)
                evict(0, accs0)
                evict(1, accs1)
                accs2 = base_quarter(2, up_first=True, close=True)   # pb0-3
                evict(2, accs2)                    # DMA overlaps q3's PE
                base_quarter(3, up_first=True, close=True,
                             tc_major=True, evict_per_tc=True)       # pb4-7

            for it in range(unroll):
                body(it)

    nc.compile()
    return nc


_NC_CACHE = {}


def _get_nc(unroll=1):
    if unroll not in _NC_CACHE:
        _NC_CACHE[unroll] = _build_nc(unroll)
    return _NC_CACHE[unroll]


def _prep_in_maps(x, weight, gate_w, A_w, B_w):
    """Host-side fp16 cast + partition-major pre-tiling (all DMAs land
    contiguous per partition)."""
    xf = np.asarray(x, np.float32).reshape(TOKENS, H)
    wT = np.asarray(weight, np.float32).T.astype(np.float16)        # [H, O]
    whq = np.ascontiguousarray(
        wT.reshape(KT, P, NQ, OQ).transpose(2, 1, 0, 3)
    )                                                               # [q,p,k,o]
    acatT = np.asarray(A_w, np.float32).transpose(2, 0, 1).reshape(H, ER)
    gacatT = np.concatenate(
        [acatT, np.asarray(gate_w, np.float32).T], axis=1
    ).astype(np.float16)                                            # [H, GA]
    gah = np.ascontiguousarray(gacatT.reshape(KT, P, GA).transpose(1, 0, 2))
    bcat = np.ascontiguousarray(
        (np.asarray(B_w, np.float32).transpose(0, 2, 1).reshape(ER, O)
         * LORA_ALPHA).astype(np.float16)
    )
    shared = {"whq": whq, "gah": gah, "bcat": bcat}
    in_maps = []
    for c in range(NCORES):
        xTc = xf[c * T : (c + 1) * T, :].T.astype(np.float16)       # [H, T]
        xhc = np.ascontiguousarray(xTc.reshape(KT, P, T).transpose(1, 0, 2))
        in_maps.append({"xh": xhc, **shared})
    return in_maps


def _unpack_out(res):
    outs = []
    for c in range(NCORES):
        arr = res.results[c]["out"]                  # [q, p, tc, oq] fp16
        full = arr.transpose(2, 1, 0, 3).reshape(T, O)
        outs.append(full.astype(np.float32))
    return np.concatenate(outs, axis=0).reshape(B, S, O)


def kernel(x, weight, gate_w, A_w, B_w, _trace=False, **_ignored):
    in_maps = _prep_in_maps(x, weight, gate_w, A_w, B_w)
    nc = _get_nc()
    res = bass_utils.run_bass_kernel_spmd(
        nc, in_maps, core_ids=list(range(NCORES)), trace=_trace
    )
    full = _unpack_out(res)
    if _trace:
        kernel.last_result = res
    return full
